# revision 1
# baseline (speedup 1.0000x reference)
"""BiLSTM-CRF loss kernel for 8x Trainium2 NeuronCores (Bass/Tile).

Sharding: data-parallel over batch (16 sentences per core). Each core runs the
identical SPMD program: embedding gather -> 2 BiLSTM layers (fwd+bwd scans
interleaved per tick) -> emissions -> CRF forward algorithm (exp-space with
periodic rescaling) + path-score numerator. Host sums the per-core partials.

Math notes (per-core, b=16, batch rows on partitions):
 - tanh(x) = 2*sigmoid(2x) - 1 everywhere, so one Sigmoid activation covers all
   four gates.  With h~ = h/2 and c~ = c/2:
     s = sigmoid(z'), z' row-scaled so s_g = sigmoid(2 z_g)
     u  = (s_g - 0.5) * s_i          ( = i*g/2 )
     c~ = s_f * c~_prev + u
     h~ = (sigmoid(4 c~) - 0.5) * s_o
   The factors of 2 are folded into the weights host-side.
 - CRF forward pass runs in exp space:  a_t = (Eexp^T a_{t-1}) .* exp(em_t),
   Eexp = exp(trans), with a partition-sum rescale every RESCALE steps whose
   log is accumulated.  logZ = ln(sum_j a_T exp(end_j)) + sum(ln rescales).
"""

import sys

sys.path.insert(0, "/opt/trn_rl_repo")

import contextlib

import numpy as np
import ml_dtypes

import concourse.bass as bass
import concourse.tile as tile
from concourse import bacc, mybir
from concourse.masks import make_identity
from concourse.bass_utils import run_bass_kernel_spmd

F32 = mybir.dt.float32
F32R = mybir.dt.float32r
BF16 = mybir.dt.bfloat16
I16 = mybir.dt.int16
AF = mybir.ActivationFunctionType
OP = mybir.AluOpType

NCORES = 8
B, T, E, H, K, V = 128, 512, 128, 128, 20, 30000
G4 = 4 * H          # 512
BL = B // NCORES    # 16 sentences per core
RESCALE = 8


def _mm(nc, out, lhsT, rhs, start, stop, fast=True):
    nc.tensor.matmul(out, lhsT, rhs, start=start, stop=stop)


def build(nt=T):
    """Build the SPMD program for sequence length nt (nt=T for real use)."""
    nc = bacc.Bacc("TRN2", target_bir_lowering=False, debug=False,
                   num_devices=NCORES)
    NTB = nt * BL   # flattened (t,b) count per core

    # ---- DRAM I/O ----
    embedb = nc.dram_tensor("embedb", [V, E], BF16, kind="ExternalInput")
    toks16 = nc.dram_tensor("toks16", [BL, nt], I16, kind="ExternalInput")
    tagsf = nc.dram_tensor("tagsf", [1, NTB], F32, kind="ExternalInput")  # b-major
    wihT0 = nc.dram_tensor("wihT0", [2, E, G4], F32R, kind="ExternalInput")
    whhT0 = nc.dram_tensor("whhT0", [2, H, G4], F32R, kind="ExternalInput")
    b0v = nc.dram_tensor("b0v", [2, 1, G4], F32R, kind="ExternalInput")
    wih1T = nc.dram_tensor("wih1T", [2, 2, H, G4], F32R, kind="ExternalInput")
    whh1T = nc.dram_tensor("whh1T", [2, H, G4], F32R, kind="ExternalInput")
    b1v = nc.dram_tensor("b1v", [2, 1, G4], F32R, kind="ExternalInput")
    woutT = nc.dram_tensor("woutT", [2, H, K], F32R, kind="ExternalInput")
    boutv = nc.dram_tensor("boutv", [K, 1], F32, kind="ExternalInput")
    transm = nc.dram_tensor("transm", [K, K], F32, kind="ExternalInput")
    startv = nc.dram_tensor("startv", [K, 1], F32, kind="ExternalInput")
    endv = nc.dram_tensor("endv", [K, 1], F32, kind="ExternalInput")
    outm = nc.dram_tensor("outm", [2, BL], F32, kind="ExternalOutput")

    with tile.TileContext(nc) as tc, contextlib.ExitStack() as ctx:
        big = ctx.enter_context(tc.tile_pool(name="big", bufs=1))
        wp = ctx.enter_context(tc.tile_pool(name="wp", bufs=1))
        work = ctx.enter_context(tc.tile_pool(name="work", bufs=3))
        stp = ctx.enter_context(tc.tile_pool(name="stp", bufs=2))

        # ---------------- P0: constants, weights, gather ----------------
        idx = wp.tile([128, nt], I16, tag="idx")
        nc.gpsimd.memset(idx[:], 0)
        nc.sync.dma_start(out=idx[0:BL, :], in_=toks16[:, :])

        def load_w(name, dram_ap, shape, dt=F32):
            t = wp.tile(shape, dt, tag=name)
            nc.sync.dma_start(out=t[:], in_=dram_ap)
            return t

        wih0_sb = [load_w(f"wih0_{d}", wihT0[d], [E, G4], F32R) for d in range(2)]
        whh0_sb = [load_w(f"whh0_{d}", whhT0[d], [H, G4], F32R) for d in range(2)]
        b0_sb = [load_w(f"b0_{d}", b0v[d], [1, G4], F32R) for d in range(2)]
        wih1_sb = [[load_w(f"wih1_{d}{h}", wih1T[d, h], [H, G4], F32R)
                    for h in range(2)] for d in range(2)]
        whh1_sb = [load_w(f"whh1_{d}", whh1T[d], [H, G4], F32R) for d in range(2)]
        b1_sb = [load_w(f"b1_{d}", b1v[d], [1, G4], F32R) for d in range(2)]
        wout_sb = [load_w(f"wout_{d}", woutT[d], [H, K], F32R) for d in range(2)]
        bout_sb = load_w("bout", boutv[:, :], [K, 1])
        trans_sb = load_w("trans", transm[:, :], [K, K])
        start_sb = load_w("start", startv[:, :], [K, 1])
        end_sb = load_w("end", endv[:, :], [K, 1])

        id16 = wp.tile([BL, BL], F32, tag="id16")
        make_identity(nc, id16[:])
        ones16f = wp.tile([1, BL], F32, tag="ones16f")
        nc.vector.memset(ones16f[:], 1.0)
        ones16 = wp.tile([1, BL], F32R, tag="ones16")
        nc.vector.tensor_copy(ones16[:], ones16f[:])
        ones20 = wp.tile([K, 1], F32, tag="ones20")
        nc.vector.memset(ones20[:], 1.0)
        ones2020 = wp.tile([K, K], F32, tag="ones2020")
        nc.vector.memset(ones2020[:], 1.0)
        iota20 = wp.tile([K, 1], mybir.dt.int32, tag="iota20i")
        nc.gpsimd.iota(iota20[:], pattern=[[0, 1]], base=0,
                       channel_multiplier=1)
        iota20f = wp.tile([K, 1], F32, tag="iota20f")
        nc.vector.tensor_copy(iota20f[:], iota20[:])
        eexp = wp.tile([K, K], F32, tag="eexp")
        nc.scalar.activation(eexp[:], trans_sb[:], AF.Exp)
        expstart = wp.tile([K, 1], F32, tag="expstart")
        nc.scalar.activation(expstart[:], start_sb[:], AF.Exp)
        expend = wp.tile([K, 1], F32, tag="expend")
        nc.scalar.activation(expend[:], end_sb[:], AF.Exp)

        # Embedding gather (+transpose): xg[128_E, NTB] bf16, col = t*BL+b
        import os
        xg = big.tile([128, 1, NTB], BF16, tag="bigB")
        if os.environ.get("KK_NO_GATHER"):
            nc.gpsimd.memset(xg[:], 0)
        else:
            GCH = 256  # idxs per gather (SWDGE descriptor-ring limit)
            for g in range(max(1, NTB // GCH)):
                cw = min(GCH, NTB)
                nc.gpsimd.dma_gather(
                    xg[:, :, g * cw:(g + 1) * cw], embedb[:, :],
                    idx[:, g * (cw // 16):(g + 1) * (cw // 16)],
                    cw, cw, E, transpose=True)
        xT = big.tile([128, NTB], F32R, tag="bigA")
        nc.vector.tensor_copy(xT[:], xg[:, 0, :])

        # Histories (feature-on-partition, t-major slices of width BL)
        h0T = [big.tile([H, NTB], F32R, tag=f"h0T{d}", name=f"h0T{d}")
               for d in range(2)]

        # ---------------- P1 / P2: the two BiLSTM layers ----------------
        def scan_layer(layer, hist_out):
            """One BiLSTM layer: fwd+bwd scans as two independent chains."""
            with tc.tile_pool(name=f"zp{layer}", bufs=2, space="PSUM") as zp, \
                 tc.tile_pool(name=f"tp{layer}", bufs=2, space="PSUM") as tp:
                cprev = []
                for d in range(2):
                    cp0 = stp.tile([BL, H], F32, tag=f"c{layer}{d}",
                                   name=f"c{layer}{d}")
                    nc.vector.memset(cp0[:], 0.0)
                    cprev.append(cp0)
                for n in range(nt):
                    tt = [n, nt - 1 - n]     # [fwd t, bwd t]
                    for d in range(2):
                        t_ = tt[d]
                        sl = slice(t_ * BL, (t_ + 1) * BL)
                        z = zp.tile([BL, G4], F32, tag=f"z{d}", name=f"z{d}")
                        if layer == 0:
                            _mm(nc, z[:], xT[:, sl], wih0_sb[d][:],
                                start=True, stop=False)
                        else:
                            _mm(nc, z[:], h0T[0][:, sl], wih1_sb[d][0][:],
                                start=True, stop=False)
                            _mm(nc, z[:], h0T[1][:, sl], wih1_sb[d][1][:],
                                start=False, stop=False)
                        wb = (whh0_sb, b0_sb) if layer == 0 else \
                             (whh1_sb, b1_sb)
                        _mm(nc, z[:], ones16[:], wb[1][d][:],
                            start=False, stop=(n == 0))
                        if n > 0:
                            tprev = tt[d] + (-1 if d == 0 else 1)
                            psl = slice(tprev * BL, (tprev + 1) * BL)
                            _mm(nc, z[:], hist_out[d][:, psl], wb[0][d][:],
                                start=False, stop=True)
                        # gates (one chain per direction); i,f,g sigmoid is
                        # on the critical path, o-gate sigmoid is not.
                        s = work.tile([BL, G4], F32, tag=f"s{d}",
                                      name=f"s{d}")
                        nc.scalar.activation(s[:], z[:], AF.Sigmoid)
                        si = s[:, 0 * H:1 * H]
                        sf = s[:, 1 * H:2 * H]
                        sg = s[:, 2 * H:3 * H]
                        so = s[:, 3 * H:4 * H]
                        u = work.tile([BL, H], F32, tag=f"u{d}", name=f"u{d}")
                        nc.vector.scalar_tensor_tensor(
                            u[:], sg, -0.5, si, OP.add, OP.mult)
                        fc = work.tile([BL, H], F32, tag=f"fc{d}",
                                       name=f"fc{d}")
                        nc.vector.tensor_tensor(fc[:], sf, cprev[d][:],
                                                OP.mult)
                        cnew = stp.tile([BL, H], F32, tag=f"c{layer}{d}",
                                        name=f"cn{layer}{d}")
                        nc.vector.tensor_tensor(cnew[:], fc[:], u[:], OP.add)
                        sc = work.tile([BL, H], F32, tag=f"sc{d}",
                                       name=f"sc{d}")
                        nc.scalar.activation(sc[:], cnew[:], AF.Sigmoid,
                                             scale=4.0)
                        hb = work.tile([BL, H], F32, tag=f"hb{d}",
                                       name=f"hb{d}")
                        nc.vector.scalar_tensor_tensor(
                            hb[:], sc[:], -0.5, so, OP.add, OP.mult)
                        ht = tp.tile([H, BL], F32, tag=f"ht{d}",
                                     name=f"ht{d}")
                        nc.tensor.transpose(ht[:], hb[:], id16[:])
                        nc.vector.tensor_copy(hist_out[d][:, sl], ht[:])
                        cprev[d] = cnew

        scan_layer(0, h0T)
        h1T = [big.tile([H, NTB], F32R, tag="bigA", name="h1T0"),
               big.tile([H, NTB], F32R, tag="bigB", name="h1T1")]
        scan_layer(1, h1T)

        # ---------------- P3a: emissions ----------------
        emr = big.tile([K, NTB], F32, tag="h0T0")     # b-major: col=b*nt+t
        expem = big.tile([K, NTB], F32, tag="h0T1")   # t-major: col=t*BL+b
        with tc.tile_pool(name="ep", bufs=2, space="PSUM") as ep:
            ECH = 512 if NTB % 512 == 0 else NTB
            etch = ECH // BL                          # t per chunk
            for c in range(NTB // ECH):
                pe = ep.tile([K, ECH], F32)
                sl = slice(c * ECH, (c + 1) * ECH)
                _mm(nc, pe[:], wout_sb[0][:], h1T[0][:, sl], True, False)
                _mm(nc, pe[:], wout_sb[1][:], h1T[1][:, sl], False, True)
                # write em (+bout) b-major via strided AP
                pe3 = pe.rearrange("p (t b) -> p t b", b=BL)
                emr3 = emr.rearrange("p (b t) -> p b t", b=BL)[
                    :, :, c * etch:(c + 1) * etch].rearrange("p b t -> p t b")
                nc.scalar.activation(emr3, pe3, AF.Identity, bias=bout_sb[:])
        # exp(em) in t-major layout
        emr_tm = emr.rearrange("p (b t) -> p t b", b=BL)
        expem3 = expem.rearrange("p (t b) -> p t b", b=BL)
        nc.scalar.activation(expem3, emr_tm, AF.Exp)

        # ---------------- P3b: CRF forward (denominator) ----------------
        with tc.tile_pool(name="cp", bufs=1, space="PSUM") as cp, \
             tc.tile_pool(name="sp", bufs=1, space="PSUM") as sp, \
             tc.tile_pool(name="npp", bufs=2, space="PSUM") as npp:
            # two independent half-batch chains interleave to hide latency
            NH = 2
            HB = BL // NH
            aps, logaccs, pendings = [], [], []
            for hh in range(NH):
                hs = slice(hh * HB, (hh + 1) * HB)
                a0 = stp.tile([K, HB], F32, tag=f"alpha{hh}", name=f"a0_{hh}")
                nc.vector.tensor_tensor(
                    a0[:], expem[:, hs],
                    expstart[:, 0:1].to_broadcast([K, HB]), OP.mult)
                la0 = stp.tile([1, HB], F32, tag=f"logacc{hh}",
                               name=f"la0_{hh}")
                nc.vector.memset(la0[:], 0.0)
                aps.append(a0)
                logaccs.append(la0)
                pendings.append(None)
            for t_ in range(1, nt):
                for hh in range(NH):
                    hs = slice(t_ * BL + hh * HB, t_ * BL + (hh + 1) * HB)
                    pa = cp.tile([K, HB], F32, tag=f"pa{hh}", name=f"pa{hh}")
                    _mm(nc, pa[:], eexp[:], aps[hh][:], True, True,
                        fast=False)
                    an = stp.tile([K, HB], F32, tag=f"alpha{hh}",
                                  name=f"an{hh}")
                    nc.vector.tensor_tensor(an[:], pa[:], expem[:, hs],
                                            OP.mult)
                    aps[hh] = an
                    if pendings[hh] is not None and t_ >= pendings[hh][1]:
                        asc = stp.tile([K, HB], F32, tag=f"alpha{hh}",
                                       name=f"as{hh}")
                        nc.vector.tensor_tensor(
                            asc[:], aps[hh][:], pendings[hh][0][:], OP.mult)
                        aps[hh] = asc
                        pendings[hh] = None
                    if t_ % RESCALE == 0 and t_ + 2 < nt:
                        ps = sp.tile([K, HB], F32, tag=f"ps{hh}",
                                     name=f"ps{hh}")
                        _mm(nc, ps[:], ones2020[:], aps[hh][:], True, True,
                            fast=False)
                        sinv = work.tile([K, HB], F32, tag=f"sinv{hh}",
                                         name=f"sinv{hh}")
                        nc.vector.reciprocal(sinv[:], ps[:])
                        lt = work.tile([1, HB], F32, tag=f"lt{hh}",
                                       name=f"lt{hh}")
                        nc.scalar.activation(lt[:], ps[0:1, :], AF.Ln)
                        la = stp.tile([1, HB], F32, tag=f"logacc{hh}",
                                      name=f"lan{hh}")
                        nc.vector.tensor_tensor(la[:], logaccs[hh][:], lt[:],
                                                OP.add)
                        logaccs[hh] = la
                        pendings[hh] = (sinv, t_ + 2)
            logz = work.tile([1, BL], F32, tag="logz")
            for hh in range(NH):
                if pendings[hh] is not None:
                    asc = stp.tile([K, HB], F32, tag=f"alpha{hh}",
                                   name=f"af{hh}")
                    nc.vector.tensor_tensor(asc[:], aps[hh][:],
                                            pendings[hh][0][:], OP.mult)
                    aps[hh] = asc
                aend = work.tile([K, HB], F32, tag=f"aend{hh}",
                                 name=f"aend{hh}")
                nc.vector.tensor_tensor(
                    aend[:], aps[hh][:],
                    expend[:, 0:1].to_broadcast([K, HB]), OP.mult)
                psf = sp.tile([K, HB], F32, tag=f"ps{hh}", name=f"psf{hh}")
                _mm(nc, psf[:], ones2020[:], aend[:], True, True, fast=False)
                lnf = work.tile([1, HB], F32, tag=f"lnf{hh}",
                                name=f"lnf{hh}")
                nc.scalar.activation(lnf[:], psf[0:1, :], AF.Ln)
                nc.vector.tensor_tensor(
                    logz[:, hh * HB:(hh + 1) * HB], lnf[:], logaccs[hh][:],
                    OP.add)
            nc.sync.dma_start(out=outm[1:2, :], in_=logz[:])

            # ---------------- P3c: numerator (path score) ----------------
            tags_rep = big.tile([K, NTB], F32, tag="bigA", name="tags_rep")
            nc.sync.dma_start(out=tags_rep[:],
                              in_=tagsf[0:1, :].to_broadcast([K, NTB]))
            scol = stp.tile([K, BL], F32, tag="scol")
            spl = stp.tile([K, BL], F32, tag="spl")
            for b in range(BL):
                base = b * nt
                ohb = work.tile([K, nt], F32, tag="ohb")
                nc.vector.tensor_tensor(
                    ohb[:], iota20f[:, 0:1].to_broadcast([K, nt]),
                    tags_rep[:, base:base + nt], OP.is_equal)
                s1 = npp.tile([K, nt - 1], F32)
                _mm(nc, s1[:], trans_sb[:], ohb[:, 0:nt - 1], True, True)
                qa = work.tile([K, nt - 1], F32, tag="qa")
                nc.vector.tensor_tensor(
                    qa[:], s1[:], emr[:, base + 1:base + nt], OP.add)
                dump = work.tile([K, nt - 1], F32, tag="dump")
                nc.vector.scalar_tensor_tensor(
                    dump[:], qa[:], 0.0, ohb[:, 1:nt],
                    OP.add, OP.mult, accum_out=scol[:, b:b + 1])
                t0 = work.tile([K, 1], F32, tag="t0")
                nc.vector.scalar_tensor_tensor(
                    t0[:], emr[:, base:base + 1], start_sb[:, 0:1],
                    ohb[:, 0:1], OP.add, OP.mult)
                te = work.tile([K, 1], F32, tag="te")
                nc.vector.tensor_tensor(
                    te[:], ohb[:, nt - 1:nt], end_sb[:, 0:1], OP.mult)
                nc.vector.tensor_tensor(spl[:, b:b + 1], t0[:], te[:], OP.add)
            psc = sp.tile([K, BL], F32, tag="psc")
            _mm(nc, psc[:], ones2020[:], scol[:], True, False, fast=False)
            _mm(nc, psc[:], ones2020[:], spl[:], False, True, fast=False)
            score = work.tile([1, BL], F32, tag="score")
            nc.vector.tensor_copy(score[:], psc[0:1, :])
            nc.sync.dma_start(out=outm[0:1, :], in_=score[:])

    nc.compile()
    return nc


# ---------------------------------------------------------------------------
# Host side
# ---------------------------------------------------------------------------
_CACHE = {}


def _get_nc(nt):
    if nt not in _CACHE:
        _CACHE[nt] = build(nt)
    return _CACHE[nt]


def prep_inputs(sentences, tags, embed, Wih0, Whh0, b0, Wih1, Whh1, b1,
                Wout, bout, trans, start, end, nt=T):
    """Host-side marshalling: weight transposes + power-of-2 gate rescales."""
    f32 = np.float32
    sc = np.ones((G4, 1), f32)
    sc[2 * H:3 * H] = 2.0           # g rows: tanh-via-sigmoid needs 2x

    def stack2(w, s):
        return np.stack([np.ascontiguousarray((w[d] * s).T.astype(f32))
                         for d in range(2)])

    wihT0 = stack2(Wih0, sc)                    # [2,128,512] (transposed)
    whhT0 = stack2(Whh0, 2.0 * sc)
    b0v = np.stack([(b0[d][None, :] * sc[:, 0][None, :]).astype(f32)
                    for d in range(2)])
    wih1T_full = stack2(Wih1, 2.0 * sc)         # [2,256,512]
    wih1T = wih1T_full.reshape(2, 2, H, G4)
    whh1T = stack2(Whh1, 2.0 * sc)
    b1v = np.stack([(b1[d][None, :] * sc[:, 0][None, :]).astype(f32)
                    for d in range(2)])
    woutT = np.stack([np.ascontiguousarray((2.0 * Wout[:, :H]).T.astype(f32)),
                      np.ascontiguousarray((2.0 * Wout[:, H:]).T.astype(f32))])
    shared = dict(
        embedb=np.ascontiguousarray(embed.astype(ml_dtypes.bfloat16)),
        wihT0=wihT0, whhT0=whhT0, b0v=b0v, wih1T=wih1T, whh1T=whh1T, b1v=b1v,
        woutT=woutT, boutv=bout.reshape(K, 1).astype(f32),
        transm=trans.astype(f32), startv=start.reshape(K, 1).astype(f32),
        endv=end.reshape(K, 1).astype(f32),
    )
    in_maps = []
    for c in range(NCORES):
        bsl = slice(c * BL, (c + 1) * BL)
        m = dict(shared)
        m["toks16"] = np.ascontiguousarray(
            sentences[bsl, :nt].astype(np.int16))
        m["tagsf"] = np.ascontiguousarray(
            tags[bsl, :nt].astype(f32).reshape(1, BL * nt))
        in_maps.append(m)
    return in_maps


def run(inputs_np, nt=T, trace=False):
    nc = _get_nc(nt)
    in_maps = prep_inputs(
        inputs_np["sentences"], inputs_np["tags"], inputs_np["embed"],
        inputs_np["Wih0"], inputs_np["Whh0"], inputs_np["b0"],
        inputs_np["Wih1"], inputs_np["Whh1"], inputs_np["b1"],
        inputs_np["Wout"], inputs_np["bout"], inputs_np["trans"],
        inputs_np["start"], inputs_np["end"], nt=nt)
    res = run_bass_kernel_spmd(nc, in_maps, core_ids=list(range(NCORES)),
                               trace=trace)
    score = np.concatenate([res.results[c]["outm"][0] for c in range(NCORES)])
    logz = np.concatenate([res.results[c]["outm"][1] for c in range(NCORES)])
    loss = -np.mean(score - logz)
    return np.float32(loss), res


def kernel(**inputs):
    inputs_np = {k: np.asarray(v) for k, v in inputs.items()}
    loss, _ = run(inputs_np, nt=T)
    return np.asarray(loss, dtype=np.float32)



# revision 7
# speedup vs baseline: 1.8381x; 1.8381x over previous
"""BiLSTM-CRF loss kernel for 8x Trainium2 NeuronCores (Bass/Tile).

Sharding: data-parallel over batch (16 sentences per core). Each core runs the
identical SPMD program: embedding gather -> 2 BiLSTM layers (fwd+bwd scans
interleaved per tick) -> emissions -> CRF forward/backward algorithm (exp-space
with periodic rescaling) + path-score numerator. Host sums per-core partials.

Layout notes (v2):
 - LSTM cell state lives feature-on-partition: z/s/c/h are [128, batch] tiles,
   so Act/DVE cost (free-dim elems + fixed init) is minimal and h is produced
   in exactly the [H, B] layout the next tick's matmul consumes (no transpose).
 - Gate preactivations per direction: psum tile [128, 4*BL], four 16-col block
   groups (i,f,g,o), each accumulated as bias (rank-1) + x-proj + h-proj
   matmuls in bf16 (1 PE cycle/row).
 - tanh(x) = 2*sigmoid(2x) - 1 everywhere, so one Sigmoid activation covers
   all four gates.  With h~ = h/2 and c~ = c/2 (factors of 2 folded into the
   weights host-side):
     s = sigmoid(z'), z' row-scaled so s_g = sigmoid(2 z_g)
     u  = (s_g - 0.5) * s_i          ( = i*g/2 )
     c~ = s_f * c~_prev + u
     h~ = (sigmoid(4 c~) - 0.5) * s_o
 - CRF partition function is computed bidirectionally to halve the sequential
   chain: alpha runs t=0..M-1 (exp space, a_t = e_t .* (Eexp^T a_{t-1})), beta
   runs t=T-1..M (b_t = e_t .* (Eexp b_{t+1}), includes end), then
   Z = (Eexp^T a_{M-1}) . b_M.  Partition-sum rescale every RESCALE steps,
   log accumulated.
 - Numerator (path score) runs on the otherwise-idle GPSIMD engine so it
   overlaps the latency-bound CRF chains.
"""

import sys

sys.path.insert(0, "/opt/trn_rl_repo")

import contextlib

import numpy as np
import ml_dtypes

import concourse.bass as bass
import concourse.tile as tile
from concourse import bacc, mybir
from concourse.bass_utils import run_bass_kernel_spmd

F32 = mybir.dt.float32
F32R = mybir.dt.float32r
BF16 = mybir.dt.bfloat16
I16 = mybir.dt.int16
AF = mybir.ActivationFunctionType
OP = mybir.AluOpType

NCORES = 8
B, T, E, H, K, V = 128, 512, 128, 128, 20, 30000
G4 = 4 * H          # 512
BL = B // NCORES    # 16 sentences per core
RESCALE = 8


def _mm(nc, out, lhsT, rhs, start, stop):
    nc.tensor.matmul(out, lhsT, rhs, start=start, stop=stop)


def build(nt=T):
    """Build the SPMD program for sequence length nt (nt=T for real use)."""
    nc = bacc.Bacc("TRN2", target_bir_lowering=False, debug=False,
                   num_devices=NCORES)
    NTB = nt * BL   # flattened (t,b) count per core

    # ---- DRAM I/O ----
    embedb = nc.dram_tensor("embedb", [V, E], BF16, kind="ExternalInput")
    toks16 = nc.dram_tensor("toks16", [BL, nt], I16, kind="ExternalInput")
    tagoh = nc.dram_tensor("tagoh", [K, NTB], BF16, kind="ExternalInput")  # one-hot, b-major
    wihT0 = nc.dram_tensor("wihT0", [2, E, G4], BF16, kind="ExternalInput")
    whhT0 = nc.dram_tensor("whhT0", [2, H, G4], BF16, kind="ExternalInput")
    b0v = nc.dram_tensor("b0v", [2, 1, G4], BF16, kind="ExternalInput")
    wih1T = nc.dram_tensor("wih1T", [2, 2, H, G4], BF16, kind="ExternalInput")
    whh1T = nc.dram_tensor("whh1T", [2, H, G4], BF16, kind="ExternalInput")
    b1v = nc.dram_tensor("b1v", [2, 1, G4], BF16, kind="ExternalInput")
    woutT = nc.dram_tensor("woutT", [2, H, K], BF16, kind="ExternalInput")
    boutv = nc.dram_tensor("boutv", [K, 1], F32, kind="ExternalInput")
    transm = nc.dram_tensor("transm", [K, K], F32, kind="ExternalInput")
    transb = nc.dram_tensor("transb", [K, K], BF16, kind="ExternalInput")
    transmT = nc.dram_tensor("transmT", [K, K], F32, kind="ExternalInput")
    startv = nc.dram_tensor("startv", [K, 1], F32, kind="ExternalInput")
    endv = nc.dram_tensor("endv", [K, 1], F32, kind="ExternalInput")
    outm = nc.dram_tensor("outm", [2, BL], F32, kind="ExternalOutput")

    with tile.TileContext(nc) as tc, contextlib.ExitStack() as ctx:
        big = ctx.enter_context(tc.tile_pool(name="big", bufs=1))
        wp = ctx.enter_context(tc.tile_pool(name="wp", bufs=1))
        work = ctx.enter_context(tc.tile_pool(name="work", bufs=3))
        stp = ctx.enter_context(tc.tile_pool(name="stp", bufs=2))

        # ---------------- P0: constants, weights, gather ----------------
        idx = wp.tile([128, nt], I16, tag="idx")
        nc.gpsimd.memset(idx[:], 0)
        nc.sync.dma_start(out=idx[0:BL, :], in_=toks16[:, :])

        def load_w(name, dram_ap, shape, dt=F32):
            t = wp.tile(shape, dt, tag=name)
            nc.sync.dma_start(out=t[:], in_=dram_ap)
            return t

        wih0_sb = [load_w(f"wih0_{d}", wihT0[d], [E, G4], BF16)
                   for d in range(2)]
        whh0_sb = [load_w(f"whh0_{d}", whhT0[d], [H, G4], BF16)
                   for d in range(2)]
        b0_sb = [load_w(f"b0_{d}", b0v[d], [1, G4], BF16) for d in range(2)]
        wih1_sb = [[load_w(f"wih1_{d}{h}", wih1T[d, h], [H, G4], BF16)
                    for h in range(2)] for d in range(2)]
        whh1_sb = [load_w(f"whh1_{d}", whh1T[d], [H, G4], BF16)
                   for d in range(2)]
        b1_sb = [load_w(f"b1_{d}", b1v[d], [1, G4], BF16) for d in range(2)]
        wout_sb = [load_w(f"wout_{d}", woutT[d], [H, K], BF16)
                   for d in range(2)]
        bout_sb = load_w("bout", boutv[:, :], [K, 1])
        trans_sb = load_w("trans", transm[:, :], [K, K])
        transb_sb = load_w("transb", transb[:, :], [K, K], BF16)
        transT_sb = load_w("transT", transmT[:, :], [K, K])
        start_sb = load_w("start", startv[:, :], [K, 1])
        end_sb = load_w("end", endv[:, :], [K, 1])

        ones16 = wp.tile([1, BL], BF16, tag="ones16")
        nc.vector.memset(ones16[:], 1.0)
        ones20 = wp.tile([K, 1], F32, tag="ones20")
        nc.vector.memset(ones20[:], 1.0)
        ones2020 = wp.tile([K, K], F32, tag="ones2020")
        nc.vector.memset(ones2020[:], 1.0)
        iota20 = wp.tile([K, 1], mybir.dt.int32, tag="iota20i")
        nc.gpsimd.iota(iota20[:], pattern=[[0, 1]], base=0,
                       channel_multiplier=1)
        iota20f = wp.tile([K, 1], F32, tag="iota20f")
        nc.vector.tensor_copy(iota20f[:], iota20[:])
        eexp = wp.tile([K, K], F32, tag="eexp")
        nc.scalar.activation(eexp[:], trans_sb[:], AF.Exp)
        eexpT = wp.tile([K, K], F32, tag="eexpT")
        nc.scalar.activation(eexpT[:], transT_sb[:], AF.Exp)
        expstart = wp.tile([K, 1], F32, tag="expstart")
        nc.scalar.activation(expstart[:], start_sb[:], AF.Exp)
        expend = wp.tile([K, 1], F32, tag="expend")
        nc.scalar.activation(expend[:], end_sb[:], AF.Exp)

        # Embedding gather (+transpose): xg[128_E, NTB] bf16, col = t*BL+b.
        # Chunks ordered head/tail interleaved so both scan directions can
        # start as soon as their end of the sequence has landed.
        xg = big.tile([128, 1, NTB], BF16, tag="bigX")
        GCH = 256  # idxs per gather (SWDGE descriptor-ring limit)
        ngch = max(1, NTB // GCH)
        gorder = []
        for i in range((ngch + 1) // 2):
            gorder.append(i)
            if ngch - 1 - i != i:
                gorder.append(ngch - 1 - i)
        for g in gorder:
            cw = min(GCH, NTB)
            nc.gpsimd.dma_gather(
                xg[:, :, g * cw:(g + 1) * cw], embedb[:, :],
                idx[:, g * (cw // 16):(g + 1) * (cw // 16)],
                cw, cw, E, transpose=True)

        # Histories (feature-on-partition, col = t*BL + b), bf16
        h0T = [big.tile([H, NTB], BF16, tag=f"h0T{d}", name=f"h0T{d}")
               for d in range(2)]
        h1T = [big.tile([H, NTB], BF16, tag=f"h1T{d}", name=f"h1T{d}")
               for d in range(2)]

        # ---------------- P1 / P2: the two BiLSTM layers ----------------
        def scan_layer(layer, hist_out):
            """One BiLSTM layer: fwd+bwd scans as two independent chains.

            All per-tick tiles are [feat(128), batch(BL)]; the four gate
            blocks sit side by side in a [128, 4*BL] psum tile.
            """
            wih = wih0_sb if layer == 0 else None
            whh = whh0_sb if layer == 0 else whh1_sb
            bb = b0_sb if layer == 0 else b1_sb
            with tc.tile_pool(name=f"zp{layer}", bufs=2, space="PSUM") as zp:
                cprev = [None, None]
                zs = [None, None]
                for n in range(nt):
                    tt = [n, nt - 1 - n]     # [fwd t, bwd t]
                    # --- PE: bias + x-proj for both dirs, then h-proj ---
                    for d in range(2):
                        t_ = tt[d]
                        sl = slice(t_ * BL, (t_ + 1) * BL)
                        z = zp.tile([H, 4 * BL], F32, tag=f"z{d}",
                                    name=f"z{d}")
                        zs[d] = z
                        for blk in range(4):
                            zb = z[:, blk * BL:(blk + 1) * BL]
                            bs = slice(blk * H, (blk + 1) * H)
                            _mm(nc, zb, bb[d][0:1, bs], ones16[:],
                                start=True, stop=False)
                            if layer == 0:
                                _mm(nc, zb, wih0_sb[d][:, bs],
                                    xg[:, 0, sl], start=False, stop=(n == 0))
                            else:
                                _mm(nc, zb, wih1_sb[d][0][:, bs],
                                    h0T[0][:, sl], start=False, stop=False)
                                _mm(nc, zb, wih1_sb[d][1][:, bs],
                                    h0T[1][:, sl], start=False, stop=(n == 0))
                    for d in range(2):
                        if n == 0:
                            continue
                        t_ = tt[d]
                        tprev = t_ + (-1 if d == 0 else 1)
                        psl = slice(tprev * BL, (tprev + 1) * BL)
                        z = zs[d]
                        for blk in range(4):
                            zb = z[:, blk * BL:(blk + 1) * BL]
                            bs = slice(blk * H, (blk + 1) * H)
                            _mm(nc, zb, whh[d][:, bs], hist_out[d][:, psl],
                                start=False, stop=True)
                    # --- Act: the one big sigmoid per dir ---
                    ss = []
                    for d in range(2):
                        s = work.tile([H, 4 * BL], F32, tag=f"s{d}",
                                      name=f"s{d}")
                        nc.scalar.activation(s[:], zs[d][:], AF.Sigmoid)
                        ss.append(s)
                    # --- DVE: cell update per dir ---
                    cns = []
                    for d in range(2):
                        s = ss[d]
                        si = s[:, 0 * BL:1 * BL]
                        sf = s[:, 1 * BL:2 * BL]
                        sg = s[:, 2 * BL:3 * BL]
                        u = work.tile([H, BL], F32, tag=f"u{d}", name=f"u{d}")
                        nc.vector.scalar_tensor_tensor(
                            u[:], sg, -0.5, si, OP.add, OP.mult)
                        if n == 0:
                            cns.append(u)
                            cprev[d] = u
                            continue
                        fc = work.tile([H, BL], F32, tag=f"fc{d}",
                                       name=f"fc{d}")
                        nc.vector.tensor_tensor(fc[:], sf, cprev[d][:],
                                                OP.mult)
                        cnew = stp.tile([H, BL], F32, tag=f"c{layer}{d}",
                                        name=f"cn{layer}{d}")
                        nc.vector.tensor_tensor(cnew[:], fc[:], u[:], OP.add)
                        cns.append(cnew)
                        cprev[d] = cnew
                    # --- Act: c-path sigmoid; DVE: h into history ---
                    scs = []
                    for d in range(2):
                        sc = work.tile([H, BL], F32, tag=f"sc{d}",
                                       name=f"sc{d}")
                        nc.scalar.activation(sc[:], cns[d][:], AF.Sigmoid,
                                             scale=4.0)
                        scs.append(sc)
                    for d in range(2):
                        t_ = tt[d]
                        sl = slice(t_ * BL, (t_ + 1) * BL)
                        so = ss[d][:, 3 * BL:4 * BL]
                        nc.vector.scalar_tensor_tensor(
                            hist_out[d][:, sl], scs[d][:], -0.5, so,
                            OP.add, OP.mult)

        scan_layer(0, h0T)
        scan_layer(1, h1T)

        # ---------------- P3a: emissions ----------------
        emr = big.tile([K, NTB], BF16, tag="emr")     # b-major: col=b*nt+t
        expem = big.tile([K, NTB], F32, tag="expem")  # t-major: col=t*BL+b
        with tc.tile_pool(name="ep", bufs=2, space="PSUM") as ep:
            ECH = 512 if NTB % 512 == 0 else NTB
            etch = ECH // BL                          # t per chunk
            nech = NTB // ECH
            # middle-out order: middle chunks finish first as the two scan
            # directions cross; the end chunks (which the CRF needs first)
            # are only ready at the very end anyway.
            corder = sorted(range(nech),
                            key=lambda c: -min(c, nech - 1 - c))
            for c in corder:
                pe = ep.tile([K, ECH], F32)
                sl = slice(c * ECH, (c + 1) * ECH)
                _mm(nc, pe[:], wout_sb[0][:], h1T[0][:, sl], True, False)
                _mm(nc, pe[:], wout_sb[1][:], h1T[1][:, sl], False, True)
                # write em (+bout) b-major via strided AP
                pe3 = pe.rearrange("p (t b) -> p t b", b=BL)
                emr3 = emr.rearrange("p (b t) -> p b t", b=BL)[
                    :, :, c * etch:(c + 1) * etch].rearrange("p b t -> p t b")
                nc.scalar.activation(emr3, pe3, AF.Identity, bias=bout_sb[:])
                # exp(em + bout) t-major, straight from psum
                nc.scalar.activation(expem[:, sl], pe[:], AF.Exp,
                                     bias=bout_sb[:])

        # ------- P3b: CRF partition function (bidirectional) -------
        MID = nt // 2   # alpha covers t=0..MID-1, beta covers t=MID..nt-1
        with tc.tile_pool(name="cp", bufs=1, space="PSUM") as cp, \
             tc.tile_pool(name="sp", bufs=1, space="PSUM") as sp, \
             tc.tile_pool(name="npp", bufs=2, space="PSUM") as npp, \
             tc.tile_pool(name="nwork", bufs=2) as nwork:
            # chain 0: alpha from t=0; chain 1: beta from t=nt-1
            aps, logaccs, pendings = [], [], []
            for hh in range(2):
                t0 = 0 if hh == 0 else nt - 1
                sl0 = slice(t0 * BL, (t0 + 1) * BL)
                a0 = stp.tile([K, BL], F32, tag=f"alpha{hh}", name=f"a0_{hh}")
                ini = expstart if hh == 0 else expend
                nc.vector.tensor_tensor(
                    a0[:], expem[:, sl0],
                    ini[:, 0:1].to_broadcast([K, BL]), OP.mult)
                la0 = stp.tile([1, BL], F32, tag=f"logacc{hh}",
                               name=f"la0_{hh}")
                nc.vector.memset(la0[:], 0.0)
                aps.append(a0)
                logaccs.append(la0)
                pendings.append(None)
            nsteps = [MID - 1, nt - 1 - MID]   # alpha: 1..MID-1; beta: nt-2..MID
            emat = [eexp, eexpT]

            # --- numerator setup (one-hot tags marshalled host-side) ---
            tagsb = big.tile([K, NTB], BF16, tag="tags_rep", name="tagsb")
            nc.sync.dma_start(out=tagsb[:], in_=tagoh[:, :])
            scol = stp.tile([K, BL], F32, tag="scol")
            spl = stp.tile([K, BL], F32, tag="spl")

            def num_batch(b):
                base = b * nt
                ohb = tagsb[:, base:base + nt]
                s1 = npp.tile([K, nt - 1], F32)
                _mm(nc, s1[:], transb_sb[:], ohb[:, 0:nt - 1], True, True)
                qa = nwork.tile([K, nt - 1], F32, tag="qa")
                nc.vector.tensor_tensor(
                    qa[:], s1[:], emr[:, base + 1:base + nt], OP.add)
                dump = nwork.tile([K, nt - 1], F32, tag="dump")
                nc.vector.scalar_tensor_tensor(
                    dump[:], qa[:], 0.0, ohb[:, 1:nt],
                    OP.add, OP.mult, accum_out=scol[:, b:b + 1])
                t0 = nwork.tile([K, 1], F32, tag="t0")
                nc.vector.scalar_tensor_tensor(
                    t0[:], emr[:, base:base + 1], start_sb[:, 0:1],
                    ohb[:, 0:1], OP.add, OP.mult)
                te = nwork.tile([K, 1], F32, tag="te")
                nc.vector.tensor_tensor(
                    te[:], ohb[:, nt - 1:nt], end_sb[:, 0:1], OP.mult)
                nc.vector.tensor_tensor(spl[:, b:b + 1], t0[:], te[:], OP.add)

            nbq = list(range(BL))  # numerator batches to interleave

            for step in range(1, max(nsteps) + 1):
                for hh in range(2):
                    if step > nsteps[hh]:
                        continue
                    t_ = step if hh == 0 else nt - 1 - step
                    sl = slice(t_ * BL, (t_ + 1) * BL)
                    pa = cp.tile([K, BL], F32, tag=f"pa{hh}", name=f"pa{hh}")
                    _mm(nc, pa[:], emat[hh][:], aps[hh][:], True, True)
                    an = stp.tile([K, BL], F32, tag=f"alpha{hh}",
                                  name=f"an{hh}")
                    nc.vector.tensor_tensor(an[:], pa[:], expem[:, sl],
                                            OP.mult)
                    aps[hh] = an
                    if pendings[hh] is not None and step >= pendings[hh][1]:
                        asc = stp.tile([K, BL], F32, tag=f"alpha{hh}",
                                       name=f"as{hh}")
                        nc.vector.tensor_tensor(
                            asc[:], aps[hh][:], pendings[hh][0][:], OP.mult)
                        aps[hh] = asc
                        pendings[hh] = None
                    if step % RESCALE == 0 and step + 2 < nsteps[hh]:
                        ps = sp.tile([K, BL], F32, tag=f"ps{hh}",
                                     name=f"ps{hh}")
                        _mm(nc, ps[:], ones2020[:], aps[hh][:], True, True)
                        sinv = work.tile([K, BL], F32, tag=f"sinv{hh}",
                                         name=f"sinv{hh}")
                        nc.vector.reciprocal(sinv[:], ps[:])
                        lt = work.tile([1, BL], F32, tag=f"lt{hh}",
                                       name=f"lt{hh}")
                        nc.scalar.activation(lt[:], ps[0:1, :], AF.Ln)
                        la = stp.tile([1, BL], F32, tag=f"logacc{hh}",
                                      name=f"lan{hh}")
                        nc.vector.tensor_tensor(la[:], logaccs[hh][:], lt[:],
                                                OP.add)
                        logaccs[hh] = la
                        pendings[hh] = (sinv, step + 2)
                # interleave one numerator batch every ~32 steps
                if step % 32 == 16 and nbq:
                    num_batch(nbq.pop(0))
            while nbq:
                num_batch(nbq.pop(0))

            for hh in range(2):
                if pendings[hh] is not None:
                    asc = stp.tile([K, BL], F32, tag=f"alpha{hh}",
                                   name=f"af{hh}")
                    nc.vector.tensor_tensor(asc[:], aps[hh][:],
                                            pendings[hh][0][:], OP.mult)
                    aps[hh] = asc
            # bridge: Z = (Eexp^T a_{MID-1}) . b_MID  (columnwise dot)
            pa = cp.tile([K, BL], F32, tag="pa0", name="pa_br")
            _mm(nc, pa[:], eexp[:], aps[0][:], True, True)
            w = work.tile([K, BL], F32, tag="wbr")
            nc.vector.tensor_tensor(w[:], pa[:], aps[1][:], OP.mult)
            psf = sp.tile([K, BL], F32, tag="ps0", name="psf")
            _mm(nc, psf[:], ones2020[:], w[:], True, True)
            lnf = work.tile([1, BL], F32, tag="lnf")
            nc.scalar.activation(lnf[:], psf[0:1, :], AF.Ln)
            logz = work.tile([1, BL], F32, tag="logz")
            nc.vector.tensor_tensor(logz[:], lnf[:], logaccs[0][:], OP.add)
            logz2 = work.tile([1, BL], F32, tag="logz2")
            nc.vector.tensor_tensor(logz2[:], logz[:], logaccs[1][:], OP.add)
            nc.sync.dma_start(out=outm[1:2, :], in_=logz2[:])

            # ---------------- P3c: numerator reduction ----------------
            psc = sp.tile([K, BL], F32, tag="ps1", name="psc")
            _mm(nc, psc[:], ones2020[:], scol[:], True, False)
            _mm(nc, psc[:], ones2020[:], spl[:], False, True)
            score = work.tile([1, BL], F32, tag="score")
            nc.vector.tensor_copy(score[:], psc[0:1, :])
            nc.sync.dma_start(out=outm[0:1, :], in_=score[:])

    nc.compile()
    return nc


# ---------------------------------------------------------------------------
# Host side
# ---------------------------------------------------------------------------
_CACHE = {}


def _get_nc(nt):
    if nt not in _CACHE:
        _CACHE[nt] = build(nt)
    return _CACHE[nt]


def prep_inputs(sentences, tags, embed, Wih0, Whh0, b0, Wih1, Whh1, b1,
                Wout, bout, trans, start, end, nt=T):
    """Host-side marshalling: weight transposes + power-of-2 gate rescales."""
    f32 = np.float32
    bf16 = ml_dtypes.bfloat16
    sc = np.ones((G4, 1), f32)
    sc[2 * H:3 * H] = 2.0           # g rows: tanh-via-sigmoid needs 2x

    def stack2(w, s):
        return np.stack([np.ascontiguousarray((w[d] * s).T.astype(bf16))
                         for d in range(2)])

    wihT0 = stack2(Wih0, sc)                    # [2,128,512] (transposed)
    whhT0 = stack2(Whh0, 2.0 * sc)
    b0v = np.stack([(b0[d] * sc[:, 0]).reshape(1, G4).astype(bf16)
                    for d in range(2)])
    wih1T_full = stack2(Wih1, 2.0 * sc)         # [2,256,512]
    wih1T = wih1T_full.reshape(2, 2, H, G4)
    whh1T = stack2(Whh1, 2.0 * sc)
    b1v = np.stack([(b1[d] * sc[:, 0]).reshape(1, G4).astype(bf16)
                    for d in range(2)])
    woutT = np.stack([np.ascontiguousarray((2.0 * Wout[:, :H]).T.astype(bf16)),
                      np.ascontiguousarray((2.0 * Wout[:, H:]).T.astype(bf16))])
    shared = dict(
        embedb=np.ascontiguousarray(embed.astype(bf16)),
        wihT0=wihT0, whhT0=whhT0, b0v=b0v, wih1T=wih1T, whh1T=whh1T, b1v=b1v,
        woutT=woutT, boutv=bout.reshape(K, 1).astype(f32),
        transm=trans.astype(f32), transb=trans.astype(bf16),
        transmT=np.ascontiguousarray(trans.T.astype(f32)),
        startv=start.reshape(K, 1).astype(f32),
        endv=end.reshape(K, 1).astype(f32),
    )
    in_maps = []
    for c in range(NCORES):
        bsl = slice(c * BL, (c + 1) * BL)
        m = dict(shared)
        m["toks16"] = np.ascontiguousarray(
            sentences[bsl, :nt].astype(np.int16))
        toh = (tags[bsl, :nt][:, None, :] ==
               np.arange(K)[None, :, None])          # [BL, K, nt]
        m["tagoh"] = np.ascontiguousarray(
            toh.transpose(1, 0, 2).reshape(K, BL * nt).astype(bf16))
        in_maps.append(m)
    return in_maps


def run(inputs_np, nt=T, trace=False):
    nc = _get_nc(nt)
    in_maps = prep_inputs(
        inputs_np["sentences"], inputs_np["tags"], inputs_np["embed"],
        inputs_np["Wih0"], inputs_np["Whh0"], inputs_np["b0"],
        inputs_np["Wih1"], inputs_np["Whh1"], inputs_np["b1"],
        inputs_np["Wout"], inputs_np["bout"], inputs_np["trans"],
        inputs_np["start"], inputs_np["end"], nt=nt)
    res = run_bass_kernel_spmd(nc, in_maps, core_ids=list(range(NCORES)),
                               trace=trace)
    score = np.concatenate([res.results[c]["outm"][0] for c in range(NCORES)])
    logz = np.concatenate([res.results[c]["outm"][1] for c in range(NCORES)])
    loss = -np.mean(score - logz)
    return np.float32(loss), res


def kernel(**inputs):
    inputs_np = {k: np.asarray(v) for k, v in inputs.items()}
    loss, _ = run(inputs_np, nt=T)
    return np.asarray(loss, dtype=np.float32)
